# Initial kernel scaffold
#
"""CoNE KG-embedding scoring kernel for 8 Trainium2 NeuronCores.

Computation (mode==0, tail_batch):
    nei   = neiMatrix[src]                  # [B,K] (host: index prep)
    ie    = ent_embed[src]                  # [B,D] (device gather)
    nkv   = nei_embed[nei]                  # [B,K,D] (device gather)
    q     = ie + rel_embed[rel]
    attn  = softmax(mask(q @ nkv / sqrt(D)))
    fused = w * (attn @ nkv) + (1-w) * ie,  w = sigmoid(weight_embed[src])
    t     = fused + rel_e
    score[b,n] = -sum_d |t[b,d] - ent_embed[dst[b,n], d]|   # [B,N]

Sharding: data-parallel over the batch dim (128 rows per core); embedding
tables replicated on every core.  Per-core layout keeps the batch row on the
SBUF partition axis throughout, so softmax/decoder broadcasts are pure APs.
"""

import numpy as np

import concourse.bacc as bacc
import concourse.bass as bass
import concourse.mybir as mybir
import concourse.tile as tile
from concourse.bass_utils import run_bass_kernel_spmd

P = 128            # SBUF partitions == per-core batch rows
D = 256            # embedding dim
K = 64             # neighbors
N = 256            # dst candidates per row
B = 1024           # global batch
E = 100000         # entities
R = 500            # relations
NCORES = 8
NB = 16            # dst block size (N // NB gather blocks in phase 2)

F32 = mybir.dt.float32
I32 = mybir.dt.int32

# mask offset at *unscaled* score scale; scores are divided by sqrt(D)=16
# inside the exp, so -1.6e10/16 == -1e9, matching the reference's fill value.
MASK_OFF = -1.6e10
INV_SQRT_D = 1.0 / 16.0

_PROGRAMS = {}         # iters -> compiled program
LAST_RESULT = None     # BassKernelResults of the most recent kernel() call


def _indirect_gather(nc, out_ap, table_ap, idx_ap):
    """out[p, m, :] = table[idx[p, m], :] (row gather, int32 indices)."""
    nc.gpsimd.indirect_dma_start(
        out=out_ap,
        out_offset=None,
        in_=table_ap,
        in_offset=bass.IndirectOffsetOnAxis(ap=idx_ap, axis=0),
    )


def _build_program(iters=1):
    nc = bacc.Bacc(
        "TRN2",
        target_bir_lowering=False,
        debug=False,
        enable_asserts=False,
        num_devices=NCORES,
    )

    ent = nc.dram_tensor("ent_embed", [E, D], F32, kind="ExternalInput").ap()
    nei_tab = nc.dram_tensor("nei_embed", [E, D], F32, kind="ExternalInput").ap()
    rel_tab = nc.dram_tensor("rel_embed", [R, D], F32, kind="ExternalInput").ap()
    src_i = nc.dram_tensor("src_idx", [P, 1], I32, kind="ExternalInput").ap()
    rel_i = nc.dram_tensor("rel_idx", [P, 1], I32, kind="ExternalInput").ap()
    nei_i = nc.dram_tensor("nei_idx", [P, K], I32, kind="ExternalInput").ap()
    dst_i = nc.dram_tensor("dst_idx", [P, N], I32, kind="ExternalInput").ap()
    offs_d = nc.dram_tensor("offs", [P, K], F32, kind="ExternalInput").ap()
    wraw_d = nc.dram_tensor("w_raw", [P, 1], F32, kind="ExternalInput").ap()
    out_d = nc.dram_tensor("out", [P, N], F32, kind="ExternalOutput").ap()

    with tile.TileContext(nc) as tc:
        with (
            tc.tile_pool(name="main", bufs=1) as pool,
            tc.tile_pool(name="pe", bufs=2) as pe_pool,
        ):
          # `iters` > 1 repeats the whole computation for timing-by-delta;
          # results are identical each iteration.
          for _ in range(iters):
            # ---- input index/aux loads -------------------------------------
            src_idx = pool.tile([P, 1], I32)
            nc.sync.dma_start(out=src_idx[:], in_=src_i[:])
            rel_idx = pool.tile([P, 1], I32)
            nc.sync.dma_start(out=rel_idx[:], in_=rel_i[:])
            nei_idx = pool.tile([P, K], I32)
            nc.sync.dma_start(out=nei_idx[:], in_=nei_i[:])
            dst_idx = pool.tile([P, N], I32)
            nc.sync.dma_start(out=dst_idx[:], in_=dst_i[:])
            offs = pool.tile([P, K], F32)
            nc.sync.dma_start(out=offs[:], in_=offs_d[:])
            w_raw = pool.tile([P, 1], F32)
            nc.sync.dma_start(out=w_raw[:], in_=wraw_d[:])

            # ---- phase 1: fused neighbor-attention embedding ---------------
            ie = pool.tile([P, D], F32)
            _indirect_gather(nc, ie[:], ent[:], src_idx[:, :1])
            rel_e = pool.tile([P, D], F32)
            _indirect_gather(nc, rel_e[:], rel_tab[:], rel_idx[:, :1])
            # one [P,1]-indexed indirect DMA per neighbor column (the only
            # row-gather shape the HW descriptor unroll supports)
            nkv = pool.tile([P, K, D], F32)
            for k in range(K):
                _indirect_gather(nc, nkv[:, k, :], nei_tab[:],
                                 nei_idx[:, k : k + 1])

            q = pool.tile([P, D], F32)
            nc.vector.tensor_add(out=q[:], in0=ie[:], in1=rel_e[:])

            # scores[b,k] = q[b,:] . nkv[b,k,:]
            # (tensor_tensor_reduce hard-faults on HW; use mul + reduce)
            scores = pool.tile([P, K], F32)
            dummy = pool.tile([P, D], F32)
            prod = pool.tile([P, K, D], F32)
            nc.vector.tensor_mul(
                out=prod[:], in0=nkv[:], in1=q[:, None, :].to_broadcast([P, K, D])
            )
            nc.vector.tensor_reduce(
                out=scores[:], in_=prod[:], axis=mybir.AxisListType.X,
                op=mybir.AluOpType.add,
            )
            # mask: add 0 / -1.6e10 (== -1e9 after the 1/16 scale in the exp)
            nc.vector.tensor_add(out=scores[:], in0=scores[:], in1=offs[:])

            # softmax over k, with the 1/sqrt(D) scale folded into the exp
            mx = pool.tile([P, 1], F32)
            nc.vector.tensor_reduce(
                out=mx[:], in_=scores[:], axis=mybir.AxisListType.X,
                op=mybir.AluOpType.max,
            )
            negmx = pool.tile([P, 1], F32)
            nc.vector.tensor_scalar_mul(out=negmx[:], in0=mx[:], scalar1=-INV_SQRT_D)
            p_t = pool.tile([P, K], F32)
            nc.scalar.activation(
                out=p_t[:], in_=scores[:], func=mybir.ActivationFunctionType.Exp,
                bias=negmx[:], scale=INV_SQRT_D,
            )
            denom = pool.tile([P, 1], F32)
            nc.vector.tensor_reduce(
                out=denom[:], in_=p_t[:], axis=mybir.AxisListType.X,
                op=mybir.AluOpType.add,
            )
            rcp = pool.tile([P, 1], F32)
            nc.vector.reciprocal(out=rcp[:], in_=denom[:])

            # nei_enc[b,d] = sum_k p[b,k] * nkv[b,k,d] / denom[b]
            p_bcast = p_t[:, :, None].to_broadcast([P, K, D])
            nc.vector.tensor_mul(out=nkv[:], in0=nkv[:], in1=p_bcast)
            nei_enc = pool.tile([P, D], F32)
            nc.vector.tensor_reduce(
                out=nei_enc[:],
                in_=nkv[:].rearrange("p k d -> p d k"),
                axis=mybir.AxisListType.X,
                op=mybir.AluOpType.add,
            )
            nc.vector.tensor_scalar_mul(out=nei_enc[:], in0=nei_enc[:], scalar1=rcp[:])

            # fused = w*nei_enc + (1-w)*ie = w*(nei_enc - ie) + ie;  t = fused + rel_e
            w = pool.tile([P, 1], F32)
            nc.scalar.activation(
                out=w[:], in_=w_raw[:], func=mybir.ActivationFunctionType.Sigmoid,
            )
            t_row = pool.tile([P, D], F32)
            nc.vector.tensor_sub(out=t_row[:], in0=nei_enc[:], in1=ie[:])
            nc.vector.tensor_scalar_mul(out=t_row[:], in0=t_row[:], scalar1=w[:])
            nc.vector.tensor_add(out=t_row[:], in0=t_row[:], in1=ie[:])
            nc.vector.tensor_add(out=t_row[:], in0=t_row[:], in1=rel_e[:])

            # ---- phase 2: TransE-L1 decode against gathered dst rows -------
            out_sb = pool.tile([P, N], F32)
            t_bcast = t_row[:, None, :].to_broadcast([P, NB, D])
            for nb in range(N // NB):
                pe = pe_pool.tile([P, NB, D], F32, tag="pe")
                for j in range(NB):
                    _indirect_gather(nc, pe[:, j, :], ent[:],
                                     dst_idx[:, nb * NB + j : nb * NB + j + 1])
                nc.vector.tensor_tensor(
                    out=pe[:], in0=pe[:], in1=t_bcast, op=mybir.AluOpType.subtract
                )
                # |.|-sum over d on the scalar engine (frees DVE for the
                # subtracts); the final negation is one cheap DVE pass below
                for j in range(NB):
                    nc.scalar.activation(
                        out=dummy[:],
                        in_=pe[:, j, :],
                        func=mybir.ActivationFunctionType.Abs,
                        accum_out=out_sb[:, nb * NB + j : nb * NB + j + 1],
                    )
            nc.vector.tensor_scalar_mul(out=out_sb[:], in0=out_sb[:], scalar1=-1.0)

            nc.sync.dma_start(out=out_d[:], in_=out_sb[:])

    nc.compile()
    return nc


def _get_program(iters=1):
    if iters not in _PROGRAMS:
        _PROGRAMS[iters] = _build_program(iters)
    return _PROGRAMS[iters]


def make_in_maps(src, rel, dst, ent_embed, rel_embed, nei_embed, weight_embed,
                 neiMatrix):
    in_maps = []
    for c in range(NCORES):
        sl = slice(c * P, (c + 1) * P)
        src_c = src[sl]
        nei_c = np.ascontiguousarray(neiMatrix[src_c])            # [P, K]
        offs_c = np.where(nei_c > 0, 0.0, MASK_OFF).astype(np.float32)
        in_maps.append({
            "ent_embed": ent_embed,
            "nei_embed": nei_embed,
            "rel_embed": rel_embed,
            "src_idx": src_c.reshape(P, 1).copy(),
            "rel_idx": rel[sl].reshape(P, 1).copy(),
            "nei_idx": nei_c,
            "dst_idx": np.ascontiguousarray(dst[sl]),
            "offs": offs_c,
            "w_raw": weight_embed[src_c].reshape(P, 1).astype(np.float32),
        })
    return in_maps


def kernel(src, rel, dst, mode, ent_embed, rel_embed, nei_embed, weight_embed,
           neiMatrix):
    global LAST_RESULT
    if int(mode) != 0:
        raise NotImplementedError("only mode==0 (tail_batch) is supported")

    src = np.asarray(src, dtype=np.int32)
    rel = np.asarray(rel, dtype=np.int32)
    dst = np.asarray(dst, dtype=np.int32)
    ent_embed = np.ascontiguousarray(np.asarray(ent_embed, dtype=np.float32))
    rel_embed = np.ascontiguousarray(np.asarray(rel_embed, dtype=np.float32))
    nei_embed = np.ascontiguousarray(np.asarray(nei_embed, dtype=np.float32))
    weight_embed = np.asarray(weight_embed, dtype=np.float32)
    neiMatrix = np.asarray(neiMatrix, dtype=np.int32)

    nc = _get_program()
    in_maps = make_in_maps(src, rel, dst, ent_embed, rel_embed, nei_embed,
                           weight_embed, neiMatrix)
    res = run_bass_kernel_spmd(nc, in_maps, list(range(NCORES)))
    LAST_RESULT = res
    out = np.concatenate([res.results[c]["out"] for c in range(NCORES)], axis=0)
    return out.astype(np.float32)



# revision 1
# speedup vs baseline: 1.2172x; 1.2172x over previous
"""CoNE KG-embedding scoring kernel for 8 Trainium2 NeuronCores.

Computation (mode==0, tail_batch):
    nei   = neiMatrix[src]                  # [B,K] (host: index prep)
    ie    = ent_embed[src]                  # [B,D] (device gather)
    nkv   = nei_embed[nei]                  # [B,K,D] (device gather)
    q     = ie + rel_embed[rel]
    attn  = softmax(mask(q @ nkv / sqrt(D)))
    fused = w * (attn @ nkv) + (1-w) * ie,  w = sigmoid(weight_embed[src])
    t     = fused + rel_e
    score[b,n] = -sum_d |t[b,d] - ent_embed[dst[b,n], d]|   # [B,N]

Sharding: data-parallel over the batch dim (128 rows per core); embedding
tables replicated on every core.  Per-core layout keeps the batch row on the
SBUF partition axis throughout, so softmax/decoder broadcasts are pure APs.
"""

import numpy as np

import concourse.bacc as bacc
import concourse.bass as bass
import concourse.mybir as mybir
import concourse.tile as tile
from concourse.bass_utils import run_bass_kernel_spmd

P = 128            # SBUF partitions == per-core batch rows
D = 256            # embedding dim
K = 64             # neighbors
N = 256            # dst candidates per row
B = 1024           # global batch
E = 100000         # entities
R = 500            # relations
NCORES = 8
NB = 16            # dst block size (N // NB gather blocks in phase 2)

F32 = mybir.dt.float32
I32 = mybir.dt.int32

# mask offset at *unscaled* score scale; scores are divided by sqrt(D)=16
# inside the exp, so -1.6e10/16 == -1e9, matching the reference's fill value.
MASK_OFF = -1.6e10
INV_SQRT_D = 1.0 / 16.0

_PROGRAMS = {}         # iters -> compiled program
LAST_RESULT = None     # BassKernelResults of the most recent kernel() call


def _indirect_gather(nc, out_ap, table_ap, idx_ap):
    """out[p, m, :] = table[idx[p, m], :] (row gather, int32 indices)."""
    nc.gpsimd.indirect_dma_start(
        out=out_ap,
        out_offset=None,
        in_=table_ap,
        in_offset=bass.IndirectOffsetOnAxis(ap=idx_ap, axis=0),
    )


def _build_program(iters=1):
    nc = bacc.Bacc(
        "TRN2",
        target_bir_lowering=False,
        debug=False,
        enable_asserts=False,
        num_devices=NCORES,
    )

    ent = nc.dram_tensor("ent_embed", [E, D], F32, kind="ExternalInput").ap()
    nei_tab = nc.dram_tensor("nei_embed", [E, D], F32, kind="ExternalInput").ap()
    rel_tab = nc.dram_tensor("rel_embed", [R, D], F32, kind="ExternalInput").ap()
    src_i = nc.dram_tensor("src_idx", [P, 1], I32, kind="ExternalInput").ap()
    rel_i = nc.dram_tensor("rel_idx", [P, 1], I32, kind="ExternalInput").ap()
    nei_i = nc.dram_tensor("nei_idx", [P, K], I32, kind="ExternalInput").ap()
    dst_i = nc.dram_tensor("dst_idx", [P, N], I32, kind="ExternalInput").ap()
    offs_d = nc.dram_tensor("offs", [P, K], F32, kind="ExternalInput").ap()
    wraw_d = nc.dram_tensor("w_raw", [P, 1], F32, kind="ExternalInput").ap()
    out_d = nc.dram_tensor("out", [P, N], F32, kind="ExternalOutput").ap()

    with tile.TileContext(nc) as tc:
        with (
            tc.tile_pool(name="main", bufs=1) as pool,
            tc.tile_pool(name="pe", bufs=2) as pe_pool,
        ):
          # `iters` > 1 repeats the whole computation for timing-by-delta;
          # results are identical each iteration.
          for _ in range(iters):
            # ---- input index/aux loads -------------------------------------
            src_idx = pool.tile([P, 1], I32)
            nc.sync.dma_start(out=src_idx[:], in_=src_i[:])
            rel_idx = pool.tile([P, 1], I32)
            nc.sync.dma_start(out=rel_idx[:], in_=rel_i[:])
            nei_idx = pool.tile([P, K], I32)
            nc.sync.dma_start(out=nei_idx[:], in_=nei_i[:])
            dst_idx = pool.tile([P, N], I32)
            nc.sync.dma_start(out=dst_idx[:], in_=dst_i[:])
            offs = pool.tile([P, K], F32)
            nc.sync.dma_start(out=offs[:], in_=offs_d[:])
            w_raw = pool.tile([P, 1], F32)
            nc.sync.dma_start(out=w_raw[:], in_=wraw_d[:])

            # ---- phase 1: fused neighbor-attention embedding ---------------
            ie = pool.tile([P, D], F32)
            _indirect_gather(nc, ie[:], ent[:], src_idx[:, :1])
            rel_e = pool.tile([P, D], F32)
            _indirect_gather(nc, rel_e[:], rel_tab[:], rel_idx[:, :1])
            # one [P,1]-indexed indirect DMA per neighbor column (the only
            # row-gather shape the HW descriptor unroll supports)
            nkv = pool.tile([P, K, D], F32)
            for k in range(K):
                _indirect_gather(nc, nkv[:, k, :], nei_tab[:],
                                 nei_idx[:, k : k + 1])

            q = pool.tile([P, D], F32)
            nc.vector.tensor_add(out=q[:], in0=ie[:], in1=rel_e[:])

            # scores[b,k] = q[b,:] . nkv[b,k,:]
            # (tensor_tensor_reduce hard-faults on HW; use mul + reduce)
            scores = pool.tile([P, K], F32)
            dummy = pool.tile([P, D], F32)
            prod = pool.tile([P, K, D], F32)
            nc.vector.tensor_mul(
                out=prod[:], in0=nkv[:], in1=q[:, None, :].to_broadcast([P, K, D])
            )
            nc.vector.tensor_reduce(
                out=scores[:], in_=prod[:], axis=mybir.AxisListType.X,
                op=mybir.AluOpType.add,
            )
            # mask: add 0 / -1.6e10 (== -1e9 after the 1/16 scale in the exp)
            nc.vector.tensor_add(out=scores[:], in0=scores[:], in1=offs[:])

            # softmax over k, with the 1/sqrt(D) scale folded into the exp
            mx = pool.tile([P, 1], F32)
            nc.vector.tensor_reduce(
                out=mx[:], in_=scores[:], axis=mybir.AxisListType.X,
                op=mybir.AluOpType.max,
            )
            negmx = pool.tile([P, 1], F32)
            nc.vector.tensor_scalar_mul(out=negmx[:], in0=mx[:], scalar1=-INV_SQRT_D)
            p_t = pool.tile([P, K], F32)
            nc.scalar.activation(
                out=p_t[:], in_=scores[:], func=mybir.ActivationFunctionType.Exp,
                bias=negmx[:], scale=INV_SQRT_D,
            )
            denom = pool.tile([P, 1], F32)
            nc.vector.tensor_reduce(
                out=denom[:], in_=p_t[:], axis=mybir.AxisListType.X,
                op=mybir.AluOpType.add,
            )
            rcp = pool.tile([P, 1], F32)
            nc.vector.reciprocal(out=rcp[:], in_=denom[:])

            # nei_enc[b,d] = sum_k p[b,k] * nkv[b,k,d] / denom[b]
            p_bcast = p_t[:, :, None].to_broadcast([P, K, D])
            nc.vector.tensor_mul(out=nkv[:], in0=nkv[:], in1=p_bcast)
            nei_enc = pool.tile([P, D], F32)
            nc.vector.tensor_reduce(
                out=nei_enc[:],
                in_=nkv[:].rearrange("p k d -> p d k"),
                axis=mybir.AxisListType.X,
                op=mybir.AluOpType.add,
            )
            nc.vector.tensor_scalar_mul(out=nei_enc[:], in0=nei_enc[:], scalar1=rcp[:])

            # fused = w*nei_enc + (1-w)*ie = w*(nei_enc - ie) + ie;  t = fused + rel_e
            w = pool.tile([P, 1], F32)
            nc.scalar.activation(
                out=w[:], in_=w_raw[:], func=mybir.ActivationFunctionType.Sigmoid,
            )
            t_row = pool.tile([P, D], F32)
            nc.vector.tensor_sub(out=t_row[:], in0=nei_enc[:], in1=ie[:])
            nc.vector.tensor_scalar_mul(out=t_row[:], in0=t_row[:], scalar1=w[:])
            nc.vector.tensor_add(out=t_row[:], in0=t_row[:], in1=ie[:])
            nc.vector.tensor_add(out=t_row[:], in0=t_row[:], in1=rel_e[:])

            # ---- phase 2: TransE-L1 decode against gathered dst rows -------
            out_sb = pool.tile([P, N], F32)
            t_bcast = t_row[:, None, :].to_broadcast([P, NB, D])
            for nb in range(N // NB):
                pe = pe_pool.tile([P, NB, D], F32, tag="pe")
                for j in range(NB):
                    _indirect_gather(nc, pe[:, j, :], ent[:],
                                     dst_idx[:, nb * NB + j : nb * NB + j + 1])
                nc.vector.tensor_tensor(
                    out=pe[:], in0=pe[:], in1=t_bcast, op=mybir.AluOpType.subtract
                )
                # |.|-sum over d on the scalar engine (frees DVE for the
                # subtracts); the final negation is one cheap DVE pass below
                for j in range(NB):
                    nc.scalar.activation(
                        out=dummy[:],
                        in_=pe[:, j, :],
                        func=mybir.ActivationFunctionType.Abs,
                        accum_out=out_sb[:, nb * NB + j : nb * NB + j + 1],
                    )
            nc.vector.tensor_scalar_mul(out=out_sb[:], in0=out_sb[:], scalar1=-1.0)

            nc.sync.dma_start(out=out_d[:], in_=out_sb[:])

    nc.compile()
    return nc


def _get_program(iters=1):
    if iters not in _PROGRAMS:
        _PROGRAMS[iters] = _build_program(iters)
    return _PROGRAMS[iters]


def make_in_maps(src, rel, dst, ent_embed, rel_embed, nei_embed, weight_embed,
                 neiMatrix):
    in_maps = []
    for c in range(NCORES):
        sl = slice(c * P, (c + 1) * P)
        src_c = src[sl]
        nei_c = np.ascontiguousarray(neiMatrix[src_c])            # [P, K]
        offs_c = np.where(nei_c > 0, 0.0, MASK_OFF).astype(np.float32)
        in_maps.append({
            "ent_embed": ent_embed,
            "nei_embed": nei_embed,
            "rel_embed": rel_embed,
            "src_idx": src_c.reshape(P, 1).copy(),
            "rel_idx": rel[sl].reshape(P, 1).copy(),
            "nei_idx": nei_c,
            "dst_idx": np.ascontiguousarray(dst[sl]),
            "offs": offs_c,
            "w_raw": weight_embed[src_c].reshape(P, 1).astype(np.float32),
        })
    return in_maps


def kernel(src, rel, dst, mode, ent_embed, rel_embed, nei_embed, weight_embed,
           neiMatrix):
    global LAST_RESULT
    if int(mode) != 0:
        raise NotImplementedError("only mode==0 (tail_batch) is supported")

    src = np.asarray(src, dtype=np.int32)
    rel = np.asarray(rel, dtype=np.int32)
    dst = np.asarray(dst, dtype=np.int32)
    ent_embed = np.ascontiguousarray(np.asarray(ent_embed, dtype=np.float32))
    rel_embed = np.ascontiguousarray(np.asarray(rel_embed, dtype=np.float32))
    nei_embed = np.ascontiguousarray(np.asarray(nei_embed, dtype=np.float32))
    weight_embed = np.asarray(weight_embed, dtype=np.float32)
    neiMatrix = np.asarray(neiMatrix, dtype=np.int32)

    nc = _get_program()
    in_maps = make_in_maps(src, rel, dst, ent_embed, rel_embed, nei_embed,
                           weight_embed, neiMatrix)
    res = run_bass_kernel_spmd(nc, in_maps, list(range(NCORES)))
    LAST_RESULT = res
    out = np.concatenate([res.results[c]["out"] for c in range(NCORES)], axis=0)
    return out.astype(np.float32)

